# revision 1
# baseline (speedup 1.0000x reference)
"""Trainium2 Bass kernel for BiLinearSigmoidAttention.

Reference math (per batch b, with L = length[b]):
    qn = l2norm(query), cn = l2norm(context)
    raw[q,k] = qn[q] . cn[k]            (masked: k >= L -> -1e30)
    sig = sigmoid(raw)
    den[q] = max(sum_k sig[q,k], 1)
    scores[q,k] = sig[q,k] / den[q]     (rows q >= L zeroed)
    att[q,:] = sum_k scores[q,k] * context[k,:]
    out = concat([qn, att], -1)
returns (out [B,S,2D], scores [B,S,S])

Device mapping (8 NeuronCores, pure data parallel over B=32 -> 4 per core):
  - mm1 computes scoresT [k_part, q_free] so the length mask is a
    per-partition bias fused into the ACT sigmoid.
  - denominator = ones-column matmuls sharing mm2's loaded weights,
    accumulated per q-block into tiny PSUM tiles (partition-major).
  - scores output produced by PE transposes of sigT, scaled by
    w = qmask/den during PSUM->SBUF eviction.
  - matmuls run as float32r (full-rate fp32); transposes as fp32.
"""

import numpy as np

import concourse.bacc as bacc
import concourse.mybir as mybir
import concourse.tile as tile
from concourse.bass_utils import run_bass_kernel_spmd

B, S, D = 32, 1024, 512
NCORES = 8
BPC = B // NCORES          # batches per core
P = 128                    # partitions
NT = S // P                # 8 s-tiles
ND = D // P                # 4 d-chunks
NEG = np.float32(-1e30)

F32 = mybir.dt.float32
F32R = mybir.dt.float32r
AF = mybir.ActivationFunctionType
ALU = mybir.AluOpType
AX = mybir.AxisListType


def _r(ap):
    """View an fp32 AP as float32r for full-rate PE matmuls."""
    return ap.bitcast(F32R)


def build_kernel():
    nc = bacc.Bacc("TRN2", target_bir_lowering=False, debug=False)

    q_d = nc.dram_tensor("query", [BPC, S, D], F32, kind="ExternalInput")
    c_d = nc.dram_tensor("context", [BPC, S, D], F32R, kind="ExternalInput")
    # keybias[b, p, kt] = 0 if kt*P+p < L else -1e30
    kb_d = nc.dram_tensor("keybias", [BPC, P, NT], F32, kind="ExternalInput")
    # qmask[b, p, qb] = 1 if qb*P+p < L else 0
    qm_d = nc.dram_tensor("qmask", [BPC, P, NT], F32, kind="ExternalInput")
    id_d = nc.dram_tensor("identity", [P, P], F32, kind="ExternalInput")
    idr_d = nc.dram_tensor("identity_r", [P, P], F32R, kind="ExternalInput")
    on_d = nc.dram_tensor("ones", [P, 2], F32R, kind="ExternalInput")
    out_d = nc.dram_tensor("out", [BPC, S, 2 * D], F32, kind="ExternalOutput")
    sc_d = nc.dram_tensor("scores", [BPC, S, S], F32, kind="ExternalOutput")

    with tile.TileContext(nc) as tc:
        _body(tc, q_d, c_d, kb_d, qm_d, id_d, idr_d, on_d, out_d, sc_d)
    nc.compile()
    return nc


def _body(tc, q_d, c_d, kb_d, qm_d, id_d, idr_d, on_d, out_d, sc_d):
    import os

    PHASE = int(os.environ.get("KERNEL_PHASE", "4"))
    nc = tc.nc
    from contextlib import ExitStack

    ctx = ExitStack()
    with ctx:
        const = ctx.enter_context(tc.tile_pool(name="const", bufs=1))
        qpool = ctx.enter_context(tc.tile_pool(name="q", bufs=2))
        cpool = ctx.enter_context(tc.tile_pool(name="c", bufs=2))
        tpool = ctx.enter_context(tc.tile_pool(name="t", bufs=1))
        sgpool = ctx.enter_context(tc.tile_pool(name="sg", bufs=1))
        mpool = ctx.enter_context(tc.tile_pool(name="m", bufs=2))
        spool = ctx.enter_context(tc.tile_pool(name="s", bufs=3))
        opool = ctx.enter_context(tc.tile_pool(name="o", bufs=3))
        ps1 = ctx.enter_context(tc.tile_pool(name="ps1", bufs=2, space="PSUM"))
        pst = ctx.enter_context(tc.tile_pool(name="pst", bufs=2, space="PSUM"))
        ps2 = ctx.enter_context(tc.tile_pool(name="ps2", bufs=2, space="PSUM"))
        psd = ctx.enter_context(tc.tile_pool(name="psd", bufs=2, space="PSUM"))

        ident = const.tile([P, P], F32, tag="ident")
        identr = const.tile([P, P], F32R, tag="identr")
        ones = const.tile([P, 2], F32R, tag="ones")
        nc.sync.dma_start(ident[:], id_d[:])
        nc.sync.dma_start(identr[:], idr_d[:])
        nc.sync.dma_start(ones[:], on_d[:])

        for b in range(BPC):
            # ---- load ----
            qt = qpool.tile([P, NT, D], F32, tag="qt")       # qn (in-place)
            ct = cpool.tile([P, NT, D], F32R, tag="ct")       # raw context
            kb = mpool.tile([P, NT], F32, tag="kb")
            qm = mpool.tile([P, NT], F32, tag="qm")
            nc.sync.dma_start(qt[:], q_d[b].rearrange("(t p) d -> p t d", p=P))
            nc.sync.dma_start(ct[:], c_d[b].rearrange("(t p) d -> p t d", p=P))
            nc.sync.dma_start(kb[:], kb_d[b])
            nc.sync.dma_start(qm[:], qm_d[b])

            # ---- norms ----
            ssq = mpool.tile([P, 2 * NT], F32, tag="ssq")
            inv = mpool.tile([P, 2 * NT], F32, tag="inv")
            for t in range(NT):
                scr = spool.tile([P, D], F32, tag="scr")
                nc.vector.tensor_mul(scr[:], qt[:, t], qt[:, t])
                nc.vector.reduce_sum(ssq[:, t : t + 1], scr[:], axis=AX.X)
                scr2 = spool.tile([P, D], F32, tag="scr2")
                nc.scalar.activation(
                    scr2[:], ct[:, t], AF.Square,
                    accum_out=ssq[:, NT + t : NT + t + 1],
                )
            # inv = 1/sqrt(ssq)  (norms are >0 with randn inputs)
            nrm = mpool.tile([P, 2 * NT], F32, tag="nrm")
            nc.scalar.activation(nrm[:], ssq[:], AF.Sqrt)
            nc.vector.reciprocal(inv[:], nrm[:])

            # ---- qn in place, store first half of out ----
            for t in range(NT):
                nc.vector.tensor_scalar_mul(qt[:, t], qt[:, t], inv[:, t : t + 1])
            nc.sync.dma_start(
                out_d[b, :, 0:D].rearrange("(t p) d -> p t d", p=P), qt[:]
            )

            if PHASE < 2:
                continue
            # ---- transposes: qT[d, s] and cnT[d, s] ----
            qT = tpool.tile([P, ND, S], F32R, tag="qT")
            cT = tpool.tile([P, ND, S], F32R, tag="cT")
            for t in range(NT):
                pq = pst.tile([P, ND, P], F32, tag="pt")
                pc = pst.tile([P, ND, P], F32R, tag="pt")
                for dch in range(ND):
                    nc.tensor.transpose(
                        pq[:, dch], qt[:, t, dch * P : (dch + 1) * P], ident[:]
                    )
                    nc.tensor.transpose(
                        pc[:, dch], ct[:, t, dch * P : (dch + 1) * P], identr[:]
                    )
                nc.scalar.copy(qT[:, :, t * P : (t + 1) * P], pq[:])
                nc.vector.tensor_copy(cT[:, :, t * P : (t + 1) * P], pc[:])

            if PHASE < 3:
                continue
            # ---- mm1: sigT[k, q] = sigmoid(cnT.T @ qT + keybias) ----
            sg = sgpool.tile([P, NT, S], F32R, tag="sg")
            for kt in range(NT):
                for qc in range(2):
                    acc = ps1.tile([P, 512], F32, tag="acc")
                    for dch in range(ND):
                        nc.tensor.matmul(
                            acc[:],
                            cT[:, dch, kt * P : (kt + 1) * P],
                            qT[:, dch, qc * 512 : (qc + 1) * 512],
                            start=(dch == 0),
                            stop=(dch == ND - 1),
                        )
                    # context l2-normalization folds in as the per-k scale
                    nc.scalar.activation(
                        sg[:, kt, qc * 512 : (qc + 1) * 512], acc[:],
                        AF.Sigmoid, bias=kb[:, kt : kt + 1],
                        scale=inv[:, NT + kt : NT + kt + 1],
                    )

            if PHASE < 4:
                continue
            # ---- per q-block: denominator, attended, scores out ----
            for qb in range(NT):
                att = ps2.tile([P, 512], F32, tag="att")
                dn = psd.tile([P, 2], F32, tag="dn")
                for kt in range(NT):
                    sgblk = sg[:, kt, qb * P : (qb + 1) * P]
                    nc.tensor.matmul(
                        att[:], sgblk, ct[:, kt],
                        start=(kt == 0), stop=(kt == NT - 1),
                    )
                    nc.tensor.matmul(
                        dn[:], sgblk, ones[:],
                        start=(kt == 0), stop=(kt == NT - 1),
                    )
                # w = qmask / max(den, 1)
                w = mpool.tile([P, 1], F32, tag="w")
                nc.vector.tensor_scalar_max(w[:], dn[:, 0:1], 1.0)
                nc.vector.reciprocal(w[:], w[:])
                nc.vector.tensor_mul(w[:], w[:], qm[:, qb : qb + 1])

                ao = opool.tile([P, D], F32, tag="ao")
                nc.vector.tensor_scalar_mul(ao[:], att[:], w[:])
                nc.sync.dma_start(out_d[b, qb * P : (qb + 1) * P, D : 2 * D], ao[:])

                so = opool.tile([P, S], F32, tag="so")
                for kg in range(2):
                    pt = pst.tile([P, 4, P], F32R, tag="pt")
                    for j in range(4):
                        kt = kg * 4 + j
                        nc.tensor.transpose(
                            pt[:, j], sg[:, kt, qb * P : (qb + 1) * P], identr[:]
                        )
                    eng = nc.scalar if kg == 0 else nc.vector
                    if kg == 0:
                        nc.scalar.activation(
                            so[:, kg * 512 : (kg + 1) * 512], pt[:],
                            AF.Copy, scale=w[:],
                        )
                    else:
                        nc.vector.tensor_scalar_mul(
                            so[:, kg * 512 : (kg + 1) * 512], pt[:], w[:]
                        )
                nc.sync.dma_start(sc_d[b, qb * P : (qb + 1) * P, :], so[:])


_NC_CACHE = {}


def _get_nc():
    if "nc" not in _NC_CACHE:
        _NC_CACHE["nc"] = build_kernel()
    return _NC_CACHE["nc"]


def kernel(context, query, length):
    context = np.ascontiguousarray(np.asarray(context, dtype=np.float32))
    query = np.ascontiguousarray(np.asarray(query, dtype=np.float32))
    length = np.asarray(length).astype(np.int64)

    iot = np.arange(S)
    keymask = iot[None, :] < length[:, None]                      # [B, S]
    kbH = np.where(keymask, np.float32(0.0), NEG).astype(np.float32)
    kbH = np.ascontiguousarray(kbH.reshape(B, NT, P).transpose(0, 2, 1))
    qmH = keymask.astype(np.float32)
    qmH = np.ascontiguousarray(qmH.reshape(B, NT, P).transpose(0, 2, 1))
    ident = np.eye(P, dtype=np.float32)

    in_maps = []
    for c in range(NCORES):
        sl = slice(c * BPC, (c + 1) * BPC)
        in_maps.append(
            {
                "query": np.ascontiguousarray(query[sl]),
                "context": np.ascontiguousarray(context[sl]),
                "keybias": np.ascontiguousarray(kbH[sl]),
                "qmask": np.ascontiguousarray(qmH[sl]),
                "identity": ident,
                "identity_r": ident,
                "ones": np.ones((P, 2), dtype=np.float32),
            }
        )

    nc = _get_nc()
    res = run_bass_kernel_spmd(nc, in_maps, list(range(NCORES)))
    _NC_CACHE["last_result"] = res
    out = np.concatenate([res.results[c]["out"] for c in range(NCORES)], axis=0)
    scores = np.concatenate(
        [res.results[c]["scores"] for c in range(NCORES)], axis=0
    )
    return out, scores



# revision 7
# speedup vs baseline: 1.6643x; 1.6643x over previous
"""Trainium2 Bass kernel for BiLinearSigmoidAttention (v2).

Reference math (per batch b, with L = length[b]):
    qn = l2norm(query), cn = l2norm(context)
    raw[q,k] = qn[q] . cn[k]            (masked: k >= L -> -1e30)
    sig = sigmoid(raw)
    den[q] = max(sum_k sig[q,k], 1)
    scores[q,k] = sig[q,k] / den[q]     (rows q >= L zeroed)
    att[q,:] = sum_k scores[q,k] * context[k,:]
    out = concat([qn, att], -1)
returns (out [B,S,2D], scores [B,S,S])

v2 design (8 NeuronCores, data parallel over B=32 -> 4 slots per core):
  - All compute and IO in bf16 (fp32 PSUM accumulation): halves DMA,
    enables fast weight loads, 2x DVE modes. rel-err budget 2e-2.
  - Length sparsity: only the top-left ceil(L/128)-block square of the
    score matrix is nonzero. Batches are sorted by length and dealt
    round-robin to cores so all 8 cores run ONE identical program whose
    per-slot block count is the max over that slot's 8 batches. The
    host zero-fills the rest of scores/att.
  - mm1 computes scoresT [k_part, q_free]; the key mask is a
    per-partition bias and the context l2-norm a per-partition scale,
    both fused into the ACT sigmoid eviction.
  - mm2 (att), denominator (ones columns) and the scoresT->scores PE
    transposes share one loop over (qb, kt).
  - Eviction work is spread across ACT / DVE / GPSIMD.
  - Dummy warm-up matmuls at kernel start keep the PE HAM clock-gate at
    full rate by the time real matmuls arrive.
"""

import numpy as np

import concourse.bacc as bacc
import concourse.mybir as mybir
import concourse.tile as tile
from concourse.bass_utils import run_bass_kernel_spmd

try:
    import ml_dtypes

    BF16 = np.dtype(ml_dtypes.bfloat16)
except ImportError:  # pragma: no cover
    BF16 = None

B, S, D = 32, 1024, 512
NCORES = 8
NSLOTS = B // NCORES       # 4 slots (batches) per core
P = 128                    # partitions
NT = S // P                # 8 s-tiles
ND = D // P                # 4 d-chunks
NEG = np.float32(-1e30)

F32 = mybir.dt.float32
BF = mybir.dt.bfloat16
AF = mybir.ActivationFunctionType
ALU = mybir.AluOpType
AX = mybir.AxisListType

N_WARM = 24  # PE warm-up matmuls (N=512) at kernel start


def build_kernel(slot_lbs):
    """slot_lbs: tuple of NSLOTS ints, block count (ceil(L/128)) per slot."""
    nc = bacc.Bacc("TRN2", target_bir_lowering=False, debug=False)

    qs, cs, kbs, qms, qns, atts, scs = [], [], [], [], [], [], []
    for i, lb in enumerate(slot_lbs):
        nb = lb * P
        qs.append(nc.dram_tensor(f"q{i}", [S, D], BF, kind="ExternalInput"))
        cs.append(nc.dram_tensor(f"c{i}", [nb, D], BF, kind="ExternalInput"))
        # kb[p, kt] = 0 if kt*P+p < L else -1e30 ; qm[p, qt] = 1/0
        kbs.append(nc.dram_tensor(f"kb{i}", [P, lb], F32, kind="ExternalInput"))
        qms.append(nc.dram_tensor(f"qm{i}", [P, lb], F32, kind="ExternalInput"))
        qns.append(nc.dram_tensor(f"qn{i}", [S, D], BF, kind="ExternalOutput"))
        atts.append(nc.dram_tensor(f"att{i}", [nb, D], BF, kind="ExternalOutput"))
        scs.append(nc.dram_tensor(f"sc{i}", [nb, nb], BF, kind="ExternalOutput"))
    id_d = nc.dram_tensor("ident", [P, P], BF, kind="ExternalInput")
    on_d = nc.dram_tensor("ones", [P, 2], BF, kind="ExternalInput")
    wm_d = nc.dram_tensor("warm", [P, 512], BF, kind="ExternalInput")

    with tile.TileContext(nc) as tc:
        _body(tc, slot_lbs, qs, cs, kbs, qms, qns, atts, scs, id_d, on_d, wm_d)
    nc.compile()
    return nc


def _body(tc, slot_lbs, qs, cs, kbs, qms, qns, atts, scs, id_d, on_d, wm_d):
    nc = tc.nc
    from contextlib import ExitStack

    ctx = ExitStack()
    with ctx:
        const = ctx.enter_context(tc.tile_pool(name="const", bufs=1))
        qpool = ctx.enter_context(tc.tile_pool(name="q", bufs=2))
        cpool = ctx.enter_context(tc.tile_pool(name="c", bufs=2))
        tpool = ctx.enter_context(tc.tile_pool(name="t", bufs=2))
        sgpool = ctx.enter_context(tc.tile_pool(name="sg", bufs=2))
        mpool = ctx.enter_context(tc.tile_pool(name="m", bufs=2))
        spool = ctx.enter_context(tc.tile_pool(name="s", bufs=2))
        opool = ctx.enter_context(tc.tile_pool(name="o", bufs=2))
        wpool = ctx.enter_context(tc.tile_pool(name="w", bufs=2))
        ps1 = ctx.enter_context(tc.tile_pool(name="ps1", bufs=2, space="PSUM"))
        pst = ctx.enter_context(tc.tile_pool(name="pst", bufs=2, space="PSUM"))
        ps2 = ctx.enter_context(tc.tile_pool(name="ps2", bufs=2, space="PSUM"))
        psd = ctx.enter_context(tc.tile_pool(name="psd", bufs=2, space="PSUM"))

        ident = const.tile([P, P], BF, tag="ident")
        ones = const.tile([P, 2], BF, tag="ones")
        warm = const.tile([P, 512], BF, tag="warm")
        nc.sync.dma_start(ident[:], id_d[:])
        nc.sync.dma_start(ones[:], on_d[:])
        nc.sync.dma_start(warm[:], wm_d[:])

        # ---- PE warm-up: release the HAM clock gate while DMAs load ----
        for _ in range(N_WARM):
            wp = ps2.tile([P, 512], F32, tag="att")
            nc.tensor.matmul(wp[:], ident[:], warm[:], start=True, stop=True)

        for i, lb in enumerate(slot_lbs):
            nb = lb * P
            # q-chunks of <=512 for mm1's free dim
            qchunks = [(j * 512, min(512, nb - j * 512))
                       for j in range((nb + 511) // 512)]

            # ---- load ----
            qt = qpool.tile([P, NT, D], BF, tag="qt")
            ct = cpool.tile([P, NT, D], BF, tag="ct")
            kb = mpool.tile([P, NT], F32, tag="kb")
            qm = mpool.tile([P, NT], F32, tag="qm")
            nc.sync.dma_start(qt[:], qs[i].rearrange("(t p) d -> p t d", p=P))
            nc.sync.dma_start(
                ct[:, 0:lb], cs[i].rearrange("(t p) d -> p t d", p=P)
            )
            nc.sync.dma_start(kb[:, 0:lb], kbs[i][:])
            nc.sync.dma_start(qm[:, 0:lb], qms[i][:])

            # ---- norms (ACT squares w/ accum; sqrt; DVE reciprocal) ----
            ssq = mpool.tile([P, 2 * NT], F32, tag="ssq")
            inv = mpool.tile([P, 2 * NT], F32, tag="inv")
            for t in range(NT):
                scr = spool.tile([P, D], BF, tag="scr")
                nc.scalar.activation(
                    scr[:], qt[:, t], AF.Square,
                    accum_out=ssq[:, t : t + 1],
                )
            for t in range(lb):
                scr = spool.tile([P, D], BF, tag="scr")
                nc.scalar.activation(
                    scr[:], ct[:, t], AF.Square,
                    accum_out=ssq[:, NT + t : NT + t + 1],
                )
            nrm = mpool.tile([P, 2 * NT], F32, tag="nrm")
            nc.scalar.activation(nrm[:, 0 : NT + lb], ssq[:, 0 : NT + lb], AF.Sqrt)
            nc.vector.reciprocal(inv[:, 0 : NT + lb], nrm[:, 0 : NT + lb])

            # ---- qn in place (DVE), store qn output ----
            for t in range(NT):
                nc.vector.tensor_scalar_mul(qt[:, t], qt[:, t], inv[:, t : t + 1])
            nc.sync.dma_start(
                qns[i].rearrange("(t p) d -> p t d", p=P), qt[:]
            )

            # ---- transposes: qT[d, q<nb] and cT[d, k<nb] (PE), evict on
            #      GPSIMD (qT) / DVE (cT) ----
            qT = tpool.tile([P, ND, S], BF, tag="qT")
            cT = tpool.tile([P, ND, S], BF, tag="cT")
            for t in range(lb):
                pq = pst.tile([P, ND, P], BF, tag="pt")
                for dch in range(ND):
                    nc.tensor.transpose(
                        pq[:, dch], qt[:, t, dch * P : (dch + 1) * P], ident[:]
                    )
                nc.scalar.copy(qT[:, :, t * P : (t + 1) * P], pq[:])
                pc = pst.tile([P, ND, P], BF, tag="pt")
                for dch in range(ND):
                    nc.tensor.transpose(
                        pc[:, dch], ct[:, t, dch * P : (dch + 1) * P], ident[:]
                    )
                nc.vector.tensor_copy(cT[:, :, t * P : (t + 1) * P], pc[:])

            # ---- mm1: sigT[k, q] = sigmoid(cT.T @ qT * inv_c + keybias) ----
            sg = sgpool.tile([P, NT, S], BF, tag="sg")
            for kt in range(lb):
                accs = []
                for (q0, qn_) in qchunks:
                    acc = ps1.tile([P, 512], F32, tag="acc")
                    accs.append(acc)
                for dch in range(ND):
                    for ci, (q0, qn_) in enumerate(qchunks):
                        nc.tensor.matmul(
                            accs[ci][:, 0:qn_],
                            cT[:, dch, kt * P : (kt + 1) * P],
                            qT[:, dch, q0 : q0 + qn_],
                            start=(dch == 0),
                            stop=(dch == ND - 1),
                        )
                for ci, (q0, qn_) in enumerate(qchunks):
                    nc.scalar.activation(
                        sg[:, kt, q0 : q0 + qn_], accs[ci][:, 0:qn_],
                        AF.Sigmoid, bias=kb[:, kt : kt + 1],
                        scale=inv[:, NT + kt : NT + kt + 1],
                    )

            # ---- per q-block: att, den, scores out ----
            ao_all = opool.tile([P, NT, D], BF, tag="ao")
            so_all = opool.tile([P, NT, S], BF, tag="so")
            for qb in range(lb):
                att = ps2.tile([P, 512], F32, tag="att")
                dn = psd.tile([P, 2], F32, tag="dn")
                ngrp = (lb + 3) // 4
                pts = [
                    pst.tile([P, 4, P], BF, tag="pt", name=f"pt{g}")
                    for g in range(ngrp)
                ]
                for kt in range(lb):
                    sgblk = sg[:, kt, qb * P : (qb + 1) * P]
                    nc.tensor.matmul(
                        att[:], sgblk, ct[:, kt],
                        start=(kt == 0), stop=(kt == lb - 1),
                    )
                    nc.tensor.matmul(
                        dn[:], sgblk, ones[:],
                        start=(kt == 0), stop=(kt == lb - 1),
                    )
                    nc.tensor.transpose(pts[kt // 4][:, kt % 4], sgblk, ident[:])
                # w = qmask / max(den, 1) = qmask * min(1/den, 1); den > 0
                winv = wpool.tile([P, 1], F32, tag="winv")
                w = wpool.tile([P, 1], F32, tag="w")
                nc.vector.reciprocal(winv[:], dn[:, 0:1])
                nc.gpsimd.tensor_scalar(
                    w[:], winv[:], 1.0, qm[:, qb : qb + 1],
                    op0=ALU.min, op1=ALU.mult,
                )
                nc.vector.tensor_scalar_mul(ao_all[:, qb], att[:], w[:])
                for g in range(ngrp):
                    n = min(4, lb - g * 4) * P
                    eng = nc.scalar if (g % 2 == 0) else nc.vector
                    if eng is nc.scalar:
                        nc.scalar.activation(
                            so_all[:, qb, g * 512 : g * 512 + n],
                            pts[g][:, 0 : n // P], AF.Copy, scale=w[:],
                        )
                    else:
                        nc.vector.tensor_scalar_mul(
                            so_all[:, qb, g * 512 : g * 512 + n],
                            pts[g][:, 0 : n // P], w[:],
                        )
            nc.sync.dma_start(
                atts[i].rearrange("(t p) d -> p t d", p=P), ao_all[:, 0:lb]
            )
            nc.sync.dma_start(
                scs[i].rearrange("(t p) k -> p t k", p=P),
                so_all[:, 0:lb, 0:nb],
            )


_NC_CACHE = {}


def _get_nc(slot_lbs):
    key = tuple(slot_lbs)
    if key not in _NC_CACHE:
        _NC_CACHE[key] = build_kernel(key)
    return _NC_CACHE[key]


def _plan(length):
    """Sort batches desc by length, deal rank r -> (slot r//8, core r%8)."""
    order = np.argsort(-length, kind="stable")
    slot_lbs = []
    for i in range(NSLOTS):
        lmax = int(length[order[i * NCORES]])
        for r in range(i * NCORES, (i + 1) * NCORES):
            lmax = max(lmax, int(length[order[r]]))
        slot_lbs.append(max(1, (lmax + P - 1) // P))
    return order, tuple(slot_lbs)


def kernel(context, query, length):
    context = np.asarray(context, dtype=np.float32)
    query = np.asarray(query, dtype=np.float32)
    length = np.asarray(length).astype(np.int64)

    order, slot_lbs = _plan(length)

    q_bf = query.astype(BF16)
    c_bf = context.astype(BF16)
    iot = np.arange(S)
    keymask = iot[None, :] < length[:, None]                      # [B, S]
    kbH = np.where(keymask, np.float32(0.0), NEG).astype(np.float32)
    kbH = np.ascontiguousarray(kbH.reshape(B, NT, P).transpose(0, 2, 1))
    qmH = keymask.astype(np.float32)
    qmH = np.ascontiguousarray(qmH.reshape(B, NT, P).transpose(0, 2, 1))

    in_maps = []
    for c in range(NCORES):
        m = {
            "ident": np.eye(P, dtype=np.float32).astype(BF16),
            "ones": np.ones((P, 2), dtype=np.float32).astype(BF16),
            "warm": np.ones((P, 512), dtype=np.float32).astype(BF16),
        }
        for i, lb in enumerate(slot_lbs):
            b = int(order[i * NCORES + c])
            nb = lb * P
            m[f"q{i}"] = np.ascontiguousarray(q_bf[b])
            m[f"c{i}"] = np.ascontiguousarray(c_bf[b, :nb])
            m[f"kb{i}"] = np.ascontiguousarray(kbH[b, :, :lb])
            m[f"qm{i}"] = np.ascontiguousarray(qmH[b, :, :lb])
        in_maps.append(m)

    nc = _get_nc(slot_lbs)
    res = run_bass_kernel_spmd(nc, in_maps, list(range(NCORES)))
    _NC_CACHE["last_result"] = res

    out = np.zeros((B, S, 2 * D), dtype=np.float32)
    scores = np.zeros((B, S, S), dtype=np.float32)
    for i, lb in enumerate(slot_lbs):
        nb = lb * P
        for c in range(NCORES):
            b = int(order[i * NCORES + c])
            r = res.results[c]
            out[b, :, 0:D] = r[f"qn{i}"].astype(np.float32)
            out[b, 0:nb, D : 2 * D] = r[f"att{i}"].astype(np.float32)
            scores[b, 0:nb, 0:nb] = r[f"sc{i}"].astype(np.float32)
    return out, scores
